# revision 1
# baseline (speedup 1.0000x reference)
"""AWQ W4A8 linear (x:[8,32,8192] f32, qweight:[8192,8192] int4-range int32,
w_scales/bias:[8192] f32) -> [8,32,8192] f32 on 8 trn2 NeuronCores.

Column-parallel sharding: qweight / w_scales / bias are split along N
(output channels) across the 8 cores; x — quantized per-token on the host
exactly as the reference does — and the per-token act_scales are
replicated. Each core computes an exact integer GEMM of
x_q [256,8192] @ qw_shard [8192,1024], applies the per-token/per-channel
dequant + bias epilogue, and writes its [256,1024] slice; the host
concatenates the slices.

Numerics: x_q in [-127,127] ships as bf16 and qw in [-8,7] ships as fp8e4
(both exactly representable), and the PE's mixed bf16 x fp8 matmul
accumulates exactly in fp32 PSUM (every product/sum is an integer < 2^24),
so the result matches the reference bit-for-bit while weight HBM traffic
drops 4x vs the int32 input encoding.

The device program is raw Bass (no TileContext) with hand-placed
semaphores: the first weight and activation pieces are issued before the
semaphore-clear barrier so the DGE spin-up overlaps program startup;
weights stream through 4 SBUF slots with ramped DMA group sizes;
remaining activations and constants ride the ACT engine's DGE queue so
they don't delay the weight stream on SP's queues; the last weight group
runs PSUM-tile-by-tile so the dequant epilogues and output stores overlap
the tail matmuls.
"""

from contextlib import ExitStack

import numpy as np

import concourse.bass as bass
import concourse.mybir as mybir
import concourse.bass_utils as bass_utils
from concourse.dt import dt as cdt

N_CORES = 8
P = 128
B, S, K, N = 8, 32, 8192, 8192
TOK = B * S                      # 256 tokens
NL = N // N_CORES                # 1024 output channels per core
KC = K // P                      # 64 contraction chunks of 128
EPS = 1e-8

W_GROUPS = [2, 4, 6, 4] + [8] * 6  # weight k-chunks per DMA group
X_GROUPS = [4, 4] + [8] * 7        # activation k-chunks per DMA piece
NSLOT = 6                          # weight SBUF slots

assert sum(W_GROUPS) == KC and sum(X_GROUPS) == KC

_cached = None


def _piece_of(c):
    acc = 0
    for i, gc in enumerate(X_GROUPS):
        if c < acc + gc:
            return i
        acc += gc
    raise ValueError(c)


def _build_nc():
    nc = bass.Bass(
        "TRN2",
        target_bir_lowering=False,
        debug=False,
        enable_asserts=False,
        num_devices=N_CORES,
    )
    dt = mybir.dt

    xq_d = nc.dram_tensor("xq", [P, KC, TOK], dt.bfloat16, kind="ExternalInput")
    qw_d = nc.dram_tensor("qw", [P, KC, NL], dt.float8e4, kind="ExternalInput")
    ws_d = nc.dram_tensor("ws", [P, NL], dt.float32, kind="ExternalInput")
    bs_d = nc.dram_tensor("bs", [P, NL], dt.float32, kind="ExternalInput")
    as_d = nc.dram_tensor("asc", [P, 2], dt.float32, kind="ExternalInput")
    out_d = nc.dram_tensor("out", [2, P, NL], dt.float32, kind="ExternalOutput")

    ctx = ExitStack()
    xq_s = ctx.enter_context(nc.sbuf_tensor("xq_s", [P, KC, TOK], dt.bfloat16))
    w_s = ctx.enter_context(nc.sbuf_tensor("w_s", [P, NSLOT, 8, NL], dt.float8e4))
    ws_s = ctx.enter_context(nc.sbuf_tensor("ws_s", [P, NL], dt.float32))
    bs_s = ctx.enter_context(nc.sbuf_tensor("bs_s", [P, NL], dt.float32))
    as_s = ctx.enter_context(nc.sbuf_tensor("as_s", [P, 2], dt.float32))
    t_s = ctx.enter_context(nc.sbuf_tensor("t_s", [P, 4, 512], dt.float32))
    o_s = ctx.enter_context(nc.sbuf_tensor("o_s", [P, 4, 512], dt.float32))

    ps = [
        ctx.enter_context(nc.psum_tensor(f"ps{i}", [P, 512], dt.float32))
        for i in range(4)  # (m,n): 00,01,10,11
    ]

    sems = {}

    def sem(name):
        sems[name] = ctx.enter_context(nc.semaphore(name))
        return sems[name]

    s_wg = [sem(f"s_wg{g}") for g in range(len(W_GROUPS))]
    s_xq = [sem(f"s_xq{i}") for i in range(len(X_GROUPS))]
    s_cst = sem("s_cst")
    s_pe = sem("s_pe")
    s_ps = [sem(f"s_ps{i}") for i in range(4)]
    s_ep = [sem(f"s_ep{i}") for i in range(4)]
    s_out = sem("s_out")
    s_dve = sem("s_dve")

    w_starts = np.cumsum([0] + W_GROUPS).tolist()
    x_starts = np.cumsum([0] + X_GROUPS).tolist()
    TILES = [(0, 0), (0, 1), (1, 0), (1, 1)]

    # Issue the critical first weight-group DMA before anything else: the
    # DGE spin-up takes ~3us, and its completion increment lands long after
    # the semaphore clears below. Prior executions fully drained (Block exit
    # drains), so clearing without a dma_reset is safe.
    nc.sync.dma_start(
        w_s[:, 0, : W_GROUPS[0], :], qw_d.ap()[:, 0 : W_GROUPS[0], :]
    ).then_inc(s_wg[0], 16)
    nc.sync.dma_start(
        xq_s[:, 0 : X_GROUPS[0], :], xq_d.ap()[:, 0 : X_GROUPS[0], :]
    ).then_inc(s_xq[0], 16)

    # Zero our semaphores up front (a previous execution of this NEFF leaves
    # them at their final values), then barrier so no engine runs ahead.
    nums = sorted(s.num for s in sems.values())
    lo = 0
    while lo < len(nums):
        hi = lo
        while hi + 1 < len(nums) and nums[hi + 1] == nums[hi] + 1:
            hi += 1
        nc.gpsimd.sem_clear(range(nums[lo], nums[hi] + 1))
        lo = hi + 1
    nc.all_engine_barrier()

    with nc.Block() as block:

        @block.sync
        def _(sync):
            for g, gc in enumerate(W_GROUPS[1:], start=1):
                if g >= NSLOT:
                    sync.wait_ge(s_pe, g - NSLOT + 1)
                c0 = w_starts[g]
                sync.dma_start(
                    w_s[:, g % NSLOT, :gc, :], qw_d.ap()[:, c0 : c0 + gc, :]
                ).then_inc(s_wg[g], 16)
            # stores for tiles 0 and 2
            for idx in (0, 2):
                m, n = TILES[idx]
                sync.wait_ge(s_ep[idx], 1)
                sync.dma_start(
                    out_d.ap()[m][:, 512 * n : 512 * (n + 1)], o_s[:, idx, :]
                ).then_inc(s_out, 16)

        @block.scalar
        def _(scalar):
            def xq_dma(i):
                xo, xc = x_starts[i], X_GROUPS[i]
                scalar.dma_start(
                    xq_s[:, xo : xo + xc, :], xq_d.ap()[:, xo : xo + xc, :]
                ).then_inc(s_xq[i], 16)

            for i in (1, 2, 3):
                xq_dma(i)
            scalar.dma_start(as_s[:], as_d.ap()).then_inc(s_cst, 16)
            scalar.dma_start(ws_s[:], ws_d.ap()).then_inc(s_cst, 16)
            scalar.dma_start(bs_s[:], bs_d.ap()).then_inc(s_cst, 16)
            for i in range(4, len(X_GROUPS)):
                xq_dma(i)
            for idx in (1, 3):
                m, n = TILES[idx]
                scalar.wait_ge(s_ep[idx], 1)
                scalar.dma_start(
                    out_d.ap()[m][:, 512 * n : 512 * (n + 1)], o_s[:, idx, :]
                ).then_inc(s_out, 16)

        @block.tensor
        def _(tensor):
            cur_piece = -1

            def mm(c, m, n, idx=None, inc_pe=False):
                nonlocal cur_piece
                pc = _piece_of(c)
                if pc != cur_piece:
                    tensor.wait_ge(s_xq[pc], 16)
                    cur_piece = pc
                g = next(i for i in range(len(W_GROUPS)) if w_starts[i + 1] > c)
                inst = tensor.matmul(
                    ps[2 * m + n].ap(),
                    xq_s[:, c, P * m : P * (m + 1)],
                    w_s[:, g % NSLOT, c - w_starts[g], 512 * n : 512 * (n + 1)],
                    start=(c == 0),
                    stop=(c == KC - 1),
                )
                if idx is not None:
                    inst.then_inc(s_ps[idx], 1)
                if inc_pe:
                    inst.then_inc(s_pe, 1)

            for g, gc in enumerate(W_GROUPS[:-1]):
                tensor.wait_ge(s_wg[g], 16)
                c0 = w_starts[g]
                for j in range(gc):
                    for m in range(2):
                        for n in range(2):
                            mm(
                                c0 + j,
                                m,
                                n,
                                inc_pe=(j == gc - 1 and m == 1 and n == 1),
                            )

            # last group: tile-by-tile so epilogues overlap the tail matmuls
            g = len(W_GROUPS) - 1
            gc = W_GROUPS[g]
            c0 = w_starts[g]
            tensor.wait_ge(s_wg[g], 16)
            for idx, (m, n) in enumerate(TILES):
                for j in range(gc):
                    mm(c0 + j, m, n, idx=(idx if j == gc - 1 else None))

        @block.vector
        def _(vector):
            vector.wait_ge(s_cst, 48)
            for idx, (m, n) in enumerate(TILES):
                nsl = slice(512 * n, 512 * (n + 1))
                vector.wait_ge(s_ps[idx], 1)
                vector.scalar_tensor_tensor(
                    t_s[:, idx, :],
                    ps[2 * m + n].ap(),
                    as_s[:, m : m + 1],
                    ws_s[:, nsl],
                    mybir.AluOpType.mult,
                    mybir.AluOpType.mult,
                ).then_inc(s_dve, 1)
                # DVE is deeply pipelined: same-engine RAW needs a sem
                vector.wait_ge(s_dve, idx + 1)
                vector.tensor_add(
                    o_s[:, idx, :], t_s[:, idx, :], bs_s[:, nsl]
                ).then_inc(s_ep[idx], 1)

    return nc, ctx


def _prep_inputs(x, qweight, w_scales, bias):
    bf16 = cdt.np(mybir.dt.bfloat16)
    fp8 = cdt.np(mybir.dt.float8e4)

    x2 = np.asarray(x, dtype=np.float32).reshape(TOK, K)
    max_abs = np.max(np.abs(x2), axis=-1, keepdims=True)
    act_scales = np.maximum(max_abs / np.float32(127.0), np.float32(EPS)).astype(
        np.float32
    )
    x_q = np.clip(np.round(x2 / act_scales), -127, 127).astype(np.float32)

    # [TOK, K] -> K-major [P, KC, TOK]: xq[p, c, t] = x_q[t, c*128 + p]
    xq = np.ascontiguousarray(
        x_q.T.reshape(KC, P, TOK).transpose(1, 0, 2).astype(bf16)
    )

    # act_scales arranged per m-tile: asc[p, m] = act_scales[m*128 + p]
    asc = np.ascontiguousarray(act_scales.reshape(2, P).T.astype(np.float32))

    # int4-range weights are exactly representable in fp8 e4m3
    qw8 = np.asarray(qweight, dtype=np.int8).astype(fp8)
    w_scales = np.asarray(w_scales, dtype=np.float32)
    bias = np.asarray(bias, dtype=np.float32)

    in_maps = []
    for i in range(N_CORES):
        sl = slice(i * NL, (i + 1) * NL)
        # [K, NL] -> p-major [P, KC, NL]: qw[p, c, n] = shard[c*128 + p, n]
        shard = qw8[:, sl].reshape(KC, P, NL).transpose(1, 0, 2)
        in_maps.append(
            {
                "xq": xq,
                "qw": np.ascontiguousarray(shard),
                "ws": np.ascontiguousarray(
                    np.broadcast_to(w_scales[sl][None, :], (P, NL))
                ),
                "bs": np.ascontiguousarray(
                    np.broadcast_to(bias[sl][None, :], (P, NL))
                ),
                "asc": asc,
            }
        )
    return in_maps


def kernel(x, qweight, w_scales, bias):
    global _cached
    if _cached is None:
        _cached = _build_nc()
    nc, _ = _cached

    in_maps = _prep_inputs(x, qweight, w_scales, bias)
    res = None
    err = None
    for _ in range(3):  # retry transient device errors
        try:
            res = bass_utils.run_bass_kernel_spmd(
                nc, in_maps, core_ids=list(range(N_CORES))
            )
            break
        except Exception as e:  # noqa: BLE001
            err = e
    if res is None:
        raise err

    out = np.empty((TOK, N), dtype=np.float32)
    for i in range(N_CORES):
        out[:, i * NL : (i + 1) * NL] = res.results[i]["out"].reshape(TOK, NL)
    return out.reshape(B, S, N)



# revision 4
# speedup vs baseline: 2.5504x; 2.5504x over previous
"""AWQ W4A8 linear (x:[8,32,8192] f32, qweight:[8192,8192] int4-range int32,
w_scales/bias:[8192] f32) -> [8,32,8192] f32 on 8 trn2 NeuronCores.

Column-parallel sharding: qweight / w_scales / bias are split along N
(output channels) across the 8 cores; x — quantized per-token on the host
exactly as the reference does — and the per-token act_scales are
replicated. Each core computes an exact integer GEMM of
x_q [256,8192] @ qw_shard [8192,1024], applies the per-token/per-channel
dequant + bias epilogue, and writes its [256,1024] slice; the host
concatenates the slices.

Numerics: x_q in [-127,127] ships as bf16 and qw in [-8,7] ships as fp8e4
(both exactly representable), and the PE's mixed bf16 x fp8 matmul
accumulates exactly in fp32 PSUM (every product/sum is an integer < 2^24),
so the result matches the reference bit-for-bit while weight HBM traffic
drops 4x vs the int32 input encoding.

The device program is raw Bass (no TileContext) with hand-placed
semaphores. The PE stream is the hard floor here (256 matmuls of N=512 at
~213 ns warm = ~55 us), so the whole design keeps the PE saturated:

- Weights (8 MB) and activations (4 MB) are fully resident in SBUF - no
  slot ring, so DMA runs arbitrarily far ahead with zero backpressure.
- A burst of dummy warm-up matmuls on garbage SBUF runs during the ~5 us
  DGE spin-up, so the HAM clock-gate reaches 2.4 GHz before the first
  real matmul and the cold-rate penalty lands in dead time.
- DMA rides three queues (sync: weights, scalar: activations, vector:
  epilogue constants) with small leading groups, so chunk 0 lands ASAP
  and x pieces are never queued behind the 1 MB of scale/bias constants.
- Chunks 0-47 run interleaved across all 4 PSUM tiles (following DMA
  arrival); chunks 48-63 run tile-by-tile so each tile's dequant
  epilogue (DVE) and output store (gpsimd queue) overlap the remaining
  tail matmuls. Only the last tile's epilogue+store is exposed (~2 us).
"""

from contextlib import ExitStack

import numpy as np

import concourse.bass as bass
import concourse.mybir as mybir
import concourse.bass_utils as bass_utils
from concourse.dt import dt as cdt

N_CORES = 8
P = 128
B, S, K, N = 8, 32, 8192, 8192
TOK = B * S                      # 256 tokens
NL = N // N_CORES                # 1024 output channels per core
KC = K // P                      # 64 contraction chunks of 128
EPS = 1e-8

W_GROUPS = [1, 1, 2, 4] + [8] * 7  # weight k-chunks per DMA group
X_GROUPS = [1, 1, 2, 4] + [8] * 7  # activation k-chunks per DMA piece
PH1 = 48                           # chunks 0..PH1-1 interleave all tiles
NWARM = 24                         # warm-up dummy matmuls (N=512, ~213ns ea)

assert sum(W_GROUPS) == KC and sum(X_GROUPS) == KC

_cached = None


def _group_of(c, groups):
    acc = 0
    for i, gc in enumerate(groups):
        acc += gc
        if c < acc:
            return i
    raise ValueError(c)


def _build_nc():
    nc = bass.Bass(
        "TRN2",
        target_bir_lowering=False,
        debug=False,
        enable_asserts=False,
        num_devices=N_CORES,
    )
    dt = mybir.dt

    xq_d = nc.dram_tensor("xq", [P, KC, TOK], dt.bfloat16, kind="ExternalInput")
    qw_d = nc.dram_tensor("qw", [P, KC, NL], dt.float8e4, kind="ExternalInput")
    ws_d = nc.dram_tensor("ws", [P, NL], dt.float32, kind="ExternalInput")
    bs_d = nc.dram_tensor("bs", [P, NL], dt.float32, kind="ExternalInput")
    as_d = nc.dram_tensor("asc", [P, 2], dt.float32, kind="ExternalInput")
    out_d = nc.dram_tensor("out", [2, P, NL], dt.float32, kind="ExternalOutput")

    ctx = ExitStack()
    xq_s = ctx.enter_context(nc.sbuf_tensor("xq_s", [P, KC, TOK], dt.bfloat16))
    w_s = ctx.enter_context(nc.sbuf_tensor("w_s", [P, KC, NL], dt.float8e4))
    ws_s = ctx.enter_context(nc.sbuf_tensor("ws_s", [P, NL], dt.float32))
    bs_s = ctx.enter_context(nc.sbuf_tensor("bs_s", [P, NL], dt.float32))
    as_s = ctx.enter_context(nc.sbuf_tensor("as_s", [P, 2], dt.float32))
    t_s = ctx.enter_context(nc.sbuf_tensor("t_s", [P, 4, 512], dt.float32))
    o_s = ctx.enter_context(nc.sbuf_tensor("o_s", [P, 4, 512], dt.float32))
    # never DMA'd: garbage operands for the PE warm-up burst
    dum_s = ctx.enter_context(nc.sbuf_tensor("dum_s", [P, 512], dt.bfloat16))

    ps = [
        ctx.enter_context(nc.psum_tensor(f"ps{i}", [P, 512], dt.float32))
        for i in range(4)  # (m,n): 00,01,10,11
    ]
    ps_warm = ctx.enter_context(nc.psum_tensor("ps_warm", [P, 512], dt.float32))

    sems = {}

    def sem(name):
        sems[name] = ctx.enter_context(nc.semaphore(name))
        return sems[name]

    s_wg = sem("s_wg")    # weight groups, +16 each, in queue order
    s_xq = sem("s_xq")    # x pieces, +16 each, in queue order
    s_cst = sem("s_cst")  # ws + bs + asc, +16 each
    s_ps = sem("s_ps")    # tile accumulation complete, +1 per tile in order
    s_dve = sem("s_dve")  # DVE same-engine RAW ordering
    s_ep = sem("s_ep")    # epilogue complete, +1 per tile in order
    s_out = sem("s_out")  # output stores

    TILES = [(0, 0), (0, 1), (1, 0), (1, 1)]

    # Issue the critical first DMAs before anything else: the DGE spin-up
    # takes ~3us and runs while the program clears semaphores / warms up.
    # Prior executions fully drained (Block exit drains), so clearing the
    # semaphores below without a dma_reset is safe.
    nc.sync.dma_start(
        w_s[:, 0 : W_GROUPS[0], :], qw_d.ap()[:, 0 : W_GROUPS[0], :]
    ).then_inc(s_wg, 16)
    nc.scalar.dma_start(
        xq_s[:, 0 : X_GROUPS[0], :], xq_d.ap()[:, 0 : X_GROUPS[0], :]
    ).then_inc(s_xq, 16)
    nc.gpsimd.dma_start(ws_s[:], ws_d.ap()).then_inc(s_cst, 16)
    nc.gpsimd.dma_start(bs_s[:], bs_d.ap()).then_inc(s_cst, 16)

    # Zero our semaphores up front (a previous execution of this NEFF leaves
    # them at their final values), then barrier so no engine runs ahead.
    nums = sorted(s.num for s in sems.values())
    lo = 0
    while lo < len(nums):
        hi = lo
        while hi + 1 < len(nums) and nums[hi + 1] == nums[hi] + 1:
            hi += 1
        nc.gpsimd.sem_clear(range(nums[lo], nums[hi] + 1))
        lo = hi + 1
    nc.all_engine_barrier()

    w_starts = np.cumsum([0] + W_GROUPS).tolist()
    x_starts = np.cumsum([0] + X_GROUPS).tolist()

    with nc.Block() as block:

        @block.sync
        def _(sync):
            for g in range(1, len(W_GROUPS)):
                c0 = w_starts[g]
                sync.dma_start(
                    w_s[:, c0 : w_starts[g + 1], :],
                    qw_d.ap()[:, c0 : w_starts[g + 1], :],
                ).then_inc(s_wg, 16)

        @block.scalar
        def _(scalar):
            for i in range(1, len(X_GROUPS)):
                c0 = x_starts[i]
                scalar.dma_start(
                    xq_s[:, c0 : x_starts[i + 1], :],
                    xq_d.ap()[:, c0 : x_starts[i + 1], :],
                ).then_inc(s_xq, 16)
            scalar.dma_start(as_s[:], as_d.ap()).then_inc(s_cst, 16)

        @block.tensor
        def _(tensor):
            # Warm-up burst: garbage matmuls into a scratch PSUM bank keep
            # the PE busy through the HAM activity window while the first
            # real chunks stream in.
            for _ in range(NWARM):
                tensor.matmul(
                    ps_warm.ap(), dum_s[:, 0:P], dum_s[:], start=True, stop=True
                )

            cur_wg = -1
            cur_xp = -1

            def need(c):
                nonlocal cur_wg, cur_xp
                g = _group_of(c, W_GROUPS)
                if g > cur_wg:
                    tensor.wait_ge(s_wg, 16 * (g + 1))
                    cur_wg = g
                i = _group_of(c, X_GROUPS)
                if i > cur_xp:
                    tensor.wait_ge(s_xq, 16 * (i + 1))
                    cur_xp = i

            def mm(c, m, n, stop=False, inc=False):
                inst = tensor.matmul(
                    ps[2 * m + n].ap(),
                    xq_s[:, c, P * m : P * (m + 1)],
                    w_s[:, c, 512 * n : 512 * (n + 1)],
                    start=(c == 0),
                    stop=stop,
                )
                if inc:
                    inst.then_inc(s_ps, 1)

            # Phase 1: chunks 0..PH1-1, all 4 tiles per chunk (follows DMA)
            for c in range(PH1):
                need(c)
                for m in range(2):
                    for n in range(2):
                        mm(c, m, n)

            # Phase 2: chunks PH1..KC-1 tile-by-tile; each tile's epilogue
            # and store overlap the next tile's matmuls.
            need(KC - 1)
            for m, n in TILES:
                for c in range(PH1, KC):
                    last = c == KC - 1
                    mm(c, m, n, stop=last, inc=last)

        @block.vector
        def _(vector):
            vector.wait_ge(s_cst, 48)
            for idx, (m, n) in enumerate(TILES):
                nsl = slice(512 * n, 512 * (n + 1))
                vector.wait_ge(s_ps, idx + 1)
                vector.scalar_tensor_tensor(
                    t_s[:, idx, :],
                    ps[2 * m + n].ap(),
                    as_s[:, m : m + 1],
                    ws_s[:, nsl],
                    mybir.AluOpType.mult,
                    mybir.AluOpType.mult,
                ).then_inc(s_dve, 1)
                # DVE is deeply pipelined: same-engine RAW needs a sem
                vector.wait_ge(s_dve, idx + 1)
                vector.tensor_add(
                    o_s[:, idx, :], t_s[:, idx, :], bs_s[:, nsl]
                ).then_inc(s_ep, 1)

        @block.gpsimd
        def _(gpsimd):
            for idx, (m, n) in enumerate(TILES):
                gpsimd.wait_ge(s_ep, idx + 1)
                gpsimd.dma_start(
                    out_d.ap()[m][:, 512 * n : 512 * (n + 1)], o_s[:, idx, :]
                ).then_inc(s_out, 16)

    return nc, ctx


def _prep_inputs(x, qweight, w_scales, bias):
    bf16 = cdt.np(mybir.dt.bfloat16)
    fp8 = cdt.np(mybir.dt.float8e4)

    x2 = np.asarray(x, dtype=np.float32).reshape(TOK, K)
    max_abs = np.max(np.abs(x2), axis=-1, keepdims=True)
    act_scales = np.maximum(max_abs / np.float32(127.0), np.float32(EPS)).astype(
        np.float32
    )
    x_q = np.clip(np.round(x2 / act_scales), -127, 127).astype(np.float32)

    # [TOK, K] -> K-major [P, KC, TOK]: xq[p, c, t] = x_q[t, c*128 + p]
    xq = np.ascontiguousarray(
        x_q.T.reshape(KC, P, TOK).transpose(1, 0, 2).astype(bf16)
    )

    # act_scales arranged per m-tile: asc[p, m] = act_scales[m*128 + p]
    asc = np.ascontiguousarray(act_scales.reshape(2, P).T.astype(np.float32))

    # int4-range weights are exactly representable in fp8 e4m3
    qw8 = np.asarray(qweight, dtype=np.int8).astype(fp8)
    w_scales = np.asarray(w_scales, dtype=np.float32)
    bias = np.asarray(bias, dtype=np.float32)

    in_maps = []
    for i in range(N_CORES):
        sl = slice(i * NL, (i + 1) * NL)
        # [K, NL] -> p-major [P, KC, NL]: qw[p, c, n] = shard[c*128 + p, n]
        shard = qw8[:, sl].reshape(KC, P, NL).transpose(1, 0, 2)
        in_maps.append(
            {
                "xq": xq,
                "qw": np.ascontiguousarray(shard),
                "ws": np.ascontiguousarray(
                    np.broadcast_to(w_scales[sl][None, :], (P, NL))
                ),
                "bs": np.ascontiguousarray(
                    np.broadcast_to(bias[sl][None, :], (P, NL))
                ),
                "asc": asc,
            }
        )
    return in_maps


def kernel(x, qweight, w_scales, bias):
    global _cached
    if _cached is None:
        _cached = _build_nc()
    nc, _ = _cached

    in_maps = _prep_inputs(x, qweight, w_scales, bias)
    res = None
    err = None
    for _ in range(3):  # retry transient device errors
        try:
            res = bass_utils.run_bass_kernel_spmd(
                nc, in_maps, core_ids=list(range(N_CORES))
            )
            break
        except Exception as e:  # noqa: BLE001
            err = e
    if res is None:
        raise err

    out = np.empty((TOK, N), dtype=np.float32)
    for i in range(N_CORES):
        out[:, i * NL : (i + 1) * NL] = res.results[i]["out"].reshape(TOK, NL)
    return out.reshape(B, S, N)


# revision 8
# speedup vs baseline: 2.7715x; 1.0867x over previous
"""AWQ W4A8 linear (x:[8,32,8192] f32, qweight:[8192,8192] int4-range int32,
w_scales/bias:[8192] f32) -> [8,32,8192] f32 on 8 trn2 NeuronCores.

Column-parallel sharding: qweight / w_scales / bias are split along N
(output channels) across the 8 cores; x — quantized per-token on the host
exactly as the reference does — and the per-token act_scales are
replicated. Each core computes an exact integer GEMM of
x_q [256,8192] @ qw_shard [8192,1024], applies the per-token/per-channel
dequant + bias epilogue, and writes its [256,1024] slice; the host
concatenates the slices.

Numerics: x_q in [-127,127] ships as bf16 and qw in [-8,7] ships as fp8e4
(both exactly representable), and the PE's mixed bf16 x fp8 matmul
accumulates exactly in fp32 PSUM (every product/sum is an integer < 2^24),
so the result matches the reference bit-for-bit while weight HBM traffic
drops 4x vs the int32 input encoding.

The device program is raw Bass (no TileContext) with hand-placed
semaphores. The PE stream is the hard floor here (256 matmuls of N=512 at
~213 ns warm = ~55 us), so the whole design keeps the PE saturated:

- Weights (8 MB) and activations (4 MB) are fully resident in SBUF - no
  slot ring, so DMA runs arbitrarily far ahead with zero backpressure.
- A burst of dummy warm-up matmuls on garbage SBUF runs during the ~5 us
  DGE spin-up, so the HAM clock-gate reaches 2.4 GHz before the first
  real matmul and the cold-rate penalty lands in dead time.
- DMA rides three queues (sync: weights, scalar: activations, vector:
  epilogue constants) with small leading groups, so chunk 0 lands ASAP
  and x pieces are never queued behind the 1 MB of scale/bias constants.
- Chunks 0-47 run interleaved across all 4 PSUM tiles (following DMA
  arrival); chunks 48-63 run tile-by-tile so each tile's dequant
  epilogue (DVE) and output store (gpsimd queue) overlap the remaining
  tail matmuls. Only the last tile's epilogue+store is exposed (~2 us).
"""

from contextlib import ExitStack

import numpy as np

import concourse.bass as bass
import concourse.mybir as mybir
import concourse.bass_utils as bass_utils
from concourse.dt import dt as cdt

N_CORES = 8
P = 128
B, S, K, N = 8, 32, 8192, 8192
TOK = B * S                      # 256 tokens
NL = N // N_CORES                # 1024 output channels per core
KC = K // P                      # 64 contraction chunks of 128
EPS = 1e-8

W_GROUPS = [1, 1, 2, 4] + [8] * 7  # weight k-chunks per DMA group
X_GROUPS = [1, 1, 2, 4] + [8] * 7  # activation k-chunks per DMA piece
PH1 = 48                           # chunks 0..PH1-1 interleave all tiles
NWARM = 10                         # warm-up dummy matmuls (N=512, ~213ns ea)

assert sum(W_GROUPS) == KC and sum(X_GROUPS) == KC

_cached = None


def _group_of(c, groups):
    acc = 0
    for i, gc in enumerate(groups):
        acc += gc
        if c < acc:
            return i
    raise ValueError(c)


def _build_nc():
    nc = bass.Bass(
        "TRN2",
        target_bir_lowering=False,
        debug=False,
        enable_asserts=False,
        num_devices=N_CORES,
    )
    dt = mybir.dt

    xq_d = nc.dram_tensor("xq", [P, KC, TOK], dt.bfloat16, kind="ExternalInput")
    qw_d = nc.dram_tensor("qw", [P, KC, NL], dt.float8e4, kind="ExternalInput")
    ws_d = nc.dram_tensor("ws", [P, NL], dt.float32, kind="ExternalInput")
    bs_d = nc.dram_tensor("bs", [P, NL], dt.float32, kind="ExternalInput")
    as_d = nc.dram_tensor("asc", [P, 2], dt.float32, kind="ExternalInput")
    out_d = nc.dram_tensor("out", [2, P, NL], dt.float32, kind="ExternalOutput")

    ctx = ExitStack()
    xq_s = ctx.enter_context(nc.sbuf_tensor("xq_s", [P, KC, TOK], dt.bfloat16))
    w_s = ctx.enter_context(nc.sbuf_tensor("w_s", [P, KC, NL], dt.float8e4))
    ws_s = ctx.enter_context(nc.sbuf_tensor("ws_s", [P, NL], dt.float32))
    bs_s = ctx.enter_context(nc.sbuf_tensor("bs_s", [P, NL], dt.float32))
    as_s = ctx.enter_context(nc.sbuf_tensor("as_s", [P, 2], dt.float32))
    t_s = ctx.enter_context(nc.sbuf_tensor("t_s", [P, 4, 512], dt.float32))
    o_s = ctx.enter_context(nc.sbuf_tensor("o_s", [P, 4, 512], dt.float32))
    # never DMA'd: garbage operands for the PE warm-up burst
    dum_s = ctx.enter_context(nc.sbuf_tensor("dum_s", [P, 512], dt.bfloat16))

    ps = [
        ctx.enter_context(nc.psum_tensor(f"ps{i}", [P, 512], dt.float32))
        for i in range(4)  # (m,n): 00,01,10,11
    ]
    ps_warm = ctx.enter_context(nc.psum_tensor("ps_warm", [P, 512], dt.float32))

    sems = {}

    def sem(name):
        sems[name] = ctx.enter_context(nc.semaphore(name))
        return sems[name]

    s_wg = sem("s_wg")    # weight groups, +16 each, in queue order
    s_xq = sem("s_xq")    # x pieces, +16 each, in queue order
    s_cst = sem("s_cst")  # ws + bs + asc, +16 each
    s_ps = sem("s_ps")    # tile accumulation complete, +1 per tile in order
    s_dve = sem("s_dve")  # DVE same-engine RAW ordering
    s_ep = sem("s_ep")    # epilogue complete, +1 per tile in order
    s_out = sem("s_out")  # output stores

    TILES = [(0, 0), (0, 1), (1, 0), (1, 1)]

    # Issue the critical first DMAs before anything else: the DGE spin-up
    # takes ~3us and runs while the program clears semaphores / warms up.
    # Prior executions fully drained (Block exit drains), so clearing the
    # semaphores below without a dma_reset is safe.
    nc.sync.dma_start(
        w_s[:, 0 : W_GROUPS[0], :], qw_d.ap()[:, 0 : W_GROUPS[0], :]
    ).then_inc(s_wg, 16)
    nc.scalar.dma_start(
        xq_s[:, 0 : X_GROUPS[0], :], xq_d.ap()[:, 0 : X_GROUPS[0], :]
    ).then_inc(s_xq, 16)

    # Zero our semaphores up front (a previous execution of this NEFF leaves
    # them at their final values), then barrier so no engine runs ahead.
    nums = sorted(s.num for s in sems.values())
    lo = 0
    while lo < len(nums):
        hi = lo
        while hi + 1 < len(nums) and nums[hi + 1] == nums[hi] + 1:
            hi += 1
        nc.gpsimd.sem_clear(range(nums[lo], nums[hi] + 1))
        lo = hi + 1
    nc.all_engine_barrier()

    w_starts = np.cumsum([0] + W_GROUPS).tolist()
    x_starts = np.cumsum([0] + X_GROUPS).tolist()

    with nc.Block() as block:

        @block.sync
        def _(sync):
            for g in range(1, len(W_GROUPS)):
                c0 = w_starts[g]
                sync.dma_start(
                    w_s[:, c0 : w_starts[g + 1], :],
                    qw_d.ap()[:, c0 : w_starts[g + 1], :],
                ).then_inc(s_wg, 16)
            for idx, (m, n) in enumerate(TILES):
                sync.wait_ge(s_ep, idx + 1)
                sync.dma_start(
                    out_d.ap()[m][:, 512 * n : 512 * (n + 1)], o_s[:, idx, :]
                ).then_inc(s_out, 16)

        @block.scalar
        def _(scalar):
            for i in range(1, len(X_GROUPS)):
                c0 = x_starts[i]
                scalar.dma_start(
                    xq_s[:, c0 : x_starts[i + 1], :],
                    xq_d.ap()[:, c0 : x_starts[i + 1], :],
                ).then_inc(s_xq, 16)
            scalar.dma_start(as_s[:], as_d.ap()).then_inc(s_cst, 16)
            scalar.dma_start(ws_s[:], ws_d.ap()).then_inc(s_cst, 16)
            scalar.dma_start(bs_s[:], bs_d.ap()).then_inc(s_cst, 16)

        @block.tensor
        def _(tensor):
            # Warm-up burst: garbage matmuls into a scratch PSUM bank keep
            # the PE busy through the HAM activity window while the first
            # real chunks stream in.
            for _ in range(NWARM):
                tensor.matmul(
                    ps_warm.ap(), dum_s[:, 0:P], dum_s[:], start=True, stop=True
                )

            cur_wg = -1
            cur_xp = -1

            def need(c):
                nonlocal cur_wg, cur_xp
                g = _group_of(c, W_GROUPS)
                if g > cur_wg:
                    tensor.wait_ge(s_wg, 16 * (g + 1))
                    cur_wg = g
                i = _group_of(c, X_GROUPS)
                if i > cur_xp:
                    tensor.wait_ge(s_xq, 16 * (i + 1))
                    cur_xp = i

            def mm(c, m, n, stop=False, inc=False):
                inst = tensor.matmul(
                    ps[2 * m + n].ap(),
                    xq_s[:, c, P * m : P * (m + 1)],
                    w_s[:, c, 512 * n : 512 * (n + 1)],
                    start=(c == 0),
                    stop=stop,
                )
                if inc:
                    inst.then_inc(s_ps, 1)

            # Phase 1: chunks 0..PH1-1, all 4 tiles per chunk (follows DMA)
            for c in range(PH1):
                need(c)
                for m in range(2):
                    for n in range(2):
                        mm(c, m, n)

            # Phase 2: chunks PH1..KC-1 tile-by-tile; each tile's epilogue
            # and store overlap the next tile's matmuls.
            need(KC - 1)
            for m, n in TILES:
                for c in range(PH1, KC):
                    last = c == KC - 1
                    mm(c, m, n, stop=last, inc=last)

        @block.vector
        def _(vector):
            vector.wait_ge(s_cst, 48)
            for idx, (m, n) in enumerate(TILES):
                nsl = slice(512 * n, 512 * (n + 1))
                vector.wait_ge(s_ps, idx + 1)
                vector.scalar_tensor_tensor(
                    t_s[:, idx, :],
                    ps[2 * m + n].ap(),
                    as_s[:, m : m + 1],
                    ws_s[:, nsl],
                    mybir.AluOpType.mult,
                    mybir.AluOpType.mult,
                ).then_inc(s_dve, 1)
                # DVE is deeply pipelined: same-engine RAW needs a sem
                vector.wait_ge(s_dve, idx + 1)
                vector.tensor_add(
                    o_s[:, idx, :], t_s[:, idx, :], bs_s[:, nsl]
                ).then_inc(s_ep, 1)

    return nc, ctx


def _prep_inputs(x, qweight, w_scales, bias):
    bf16 = cdt.np(mybir.dt.bfloat16)
    fp8 = cdt.np(mybir.dt.float8e4)

    x2 = np.asarray(x, dtype=np.float32).reshape(TOK, K)
    max_abs = np.max(np.abs(x2), axis=-1, keepdims=True)
    act_scales = np.maximum(max_abs / np.float32(127.0), np.float32(EPS)).astype(
        np.float32
    )
    x_q = np.clip(np.round(x2 / act_scales), -127, 127).astype(np.float32)

    # [TOK, K] -> K-major [P, KC, TOK]: xq[p, c, t] = x_q[t, c*128 + p]
    xq = np.ascontiguousarray(
        x_q.T.reshape(KC, P, TOK).transpose(1, 0, 2).astype(bf16)
    )

    # act_scales arranged per m-tile: asc[p, m] = act_scales[m*128 + p]
    asc = np.ascontiguousarray(act_scales.reshape(2, P).T.astype(np.float32))

    # int4-range weights are exactly representable in fp8 e4m3
    qw8 = np.asarray(qweight, dtype=np.int8).astype(fp8)
    w_scales = np.asarray(w_scales, dtype=np.float32)
    bias = np.asarray(bias, dtype=np.float32)

    in_maps = []
    for i in range(N_CORES):
        sl = slice(i * NL, (i + 1) * NL)
        # [K, NL] -> p-major [P, KC, NL]: qw[p, c, n] = shard[c*128 + p, n]
        shard = qw8[:, sl].reshape(KC, P, NL).transpose(1, 0, 2)
        in_maps.append(
            {
                "xq": xq,
                "qw": np.ascontiguousarray(shard),
                "ws": np.ascontiguousarray(
                    np.broadcast_to(w_scales[sl][None, :], (P, NL))
                ),
                "bs": np.ascontiguousarray(
                    np.broadcast_to(bias[sl][None, :], (P, NL))
                ),
                "asc": asc,
            }
        )
    return in_maps


def kernel(x, qweight, w_scales, bias):
    global _cached
    if _cached is None:
        _cached = _build_nc()
    nc, _ = _cached

    in_maps = _prep_inputs(x, qweight, w_scales, bias)
    res = None
    err = None
    for _ in range(3):  # retry transient device errors
        try:
            res = bass_utils.run_bass_kernel_spmd(
                nc, in_maps, core_ids=list(range(N_CORES))
            )
            break
        except Exception as e:  # noqa: BLE001
            err = e
    if res is None:
        raise err

    out = np.empty((TOK, N), dtype=np.float32)
    for i in range(N_CORES):
        out[:, i * NL : (i + 1) * NL] = res.results[i]["out"].reshape(TOK, NL)
    return out.reshape(B, S, N)


# revision 13
# speedup vs baseline: 2.9510x; 1.0648x over previous
"""AWQ W4A8 linear (x:[8,32,8192] f32, qweight:[8192,8192] int4-range int32,
w_scales/bias:[8192] f32) -> [8,32,8192] f32 on 8 trn2 NeuronCores.

Column-parallel sharding: qweight / w_scales / bias are split along N
(output channels) across the 8 cores; x — quantized per-token on the host
exactly as the reference does — and the per-token act_scales are
replicated. Each core computes an exact integer GEMM of
x_q [256,8192] @ qw_shard [8192,1024], applies the per-token/per-channel
dequant + bias epilogue, and writes its [256,1024] slice; the host
concatenates the slices.

Numerics: x_q in [-127,127] ships as bf16 and qw in [-8,7] ships as fp8e4
(both exactly representable), and the PE's mixed bf16 x fp8 matmul
accumulates exactly in fp32 PSUM (every product/sum is an integer < 2^24),
so the result matches the reference bit-for-bit while weight HBM traffic
drops 4x vs the int32 input encoding.

The device program is raw Bass (no TileContext) with hand-placed
semaphores. The PE stream is the hard floor here (256 matmuls of N=512 at
~213 ns warm = ~55 us), so the whole design keeps the PE saturated:

- Weights (8 MB) and activations (4 MB) are fully resident in SBUF - no
  slot ring, so DMA runs arbitrarily far ahead with zero backpressure.
- A burst of dummy warm-up matmuls on garbage SBUF runs during the ~5 us
  DGE spin-up, so the HAM clock-gate reaches 2.4 GHz before the first
  real matmul and the cold-rate penalty lands in dead time.
- DMA rides three queues (sync: weights, scalar: activations, vector:
  epilogue constants) with small leading groups, so chunk 0 lands ASAP
  and x pieces are never queued behind the 1 MB of scale/bias constants.
- Chunks 0-47 run interleaved across all 4 PSUM tiles (following DMA
  arrival); chunks 48-63 run tile-by-tile so each tile's dequant
  epilogue (DVE) and output store (gpsimd queue) overlap the remaining
  tail matmuls. Only the last tile's epilogue+store is exposed (~2 us).
"""

from contextlib import ExitStack

import numpy as np

import concourse.bass as bass
import concourse.mybir as mybir
import concourse.bass_utils as bass_utils
from concourse.dt import dt as cdt

N_CORES = 8
P = 128
B, S, K, N = 8, 32, 8192, 8192
TOK = B * S                      # 256 tokens
NL = N // N_CORES                # 1024 output channels per core
KC = K // P                      # 64 contraction chunks of 128
EPS = 1e-8

W_GROUPS = [1, 1, 2, 4] + [8] * 7  # weight k-chunks per DMA group
X_GROUPS = [1, 1, 2, 4] + [8] * 7  # activation k-chunks per DMA piece
PH1 = 48                           # chunks 0..PH1-1 interleave all tiles
NWARM = 10                         # warm-up dummy matmuls (N=512, ~213ns ea)

assert sum(W_GROUPS) == KC and sum(X_GROUPS) == KC

_cached = None


def _group_of(c, groups):
    acc = 0
    for i, gc in enumerate(groups):
        acc += gc
        if c < acc:
            return i
    raise ValueError(c)


def _build_nc():
    nc = bass.Bass(
        "TRN2",
        target_bir_lowering=False,
        debug=False,
        enable_asserts=False,
        num_devices=N_CORES,
    )
    dt = mybir.dt

    xq_d = nc.dram_tensor("xq", [P, KC, TOK], dt.bfloat16, kind="ExternalInput")
    qw_d = nc.dram_tensor("qw", [P, KC, NL], dt.float8e4, kind="ExternalInput")
    ws_d = nc.dram_tensor("ws", [P, NL], dt.float32, kind="ExternalInput")
    bs_d = nc.dram_tensor("bs", [P, NL], dt.float32, kind="ExternalInput")
    as_d = nc.dram_tensor("asc", [P, 2], dt.float32, kind="ExternalInput")
    out_d = nc.dram_tensor("out", [2, P, NL], dt.float32, kind="ExternalOutput")

    ctx = ExitStack()
    xq_s = ctx.enter_context(nc.sbuf_tensor("xq_s", [P, KC, TOK], dt.bfloat16))
    w_s = ctx.enter_context(nc.sbuf_tensor("w_s", [P, KC, NL], dt.float8e4))
    ws_s = ctx.enter_context(nc.sbuf_tensor("ws_s", [P, NL], dt.float32))
    bs_s = ctx.enter_context(nc.sbuf_tensor("bs_s", [P, NL], dt.float32))
    as_s = ctx.enter_context(nc.sbuf_tensor("as_s", [P, 2], dt.float32))
    t_s = ctx.enter_context(nc.sbuf_tensor("t_s", [P, 4, 512], dt.float32))
    o_s = ctx.enter_context(nc.sbuf_tensor("o_s", [P, 4, 512], dt.float32))
    # never DMA'd: garbage operands for the PE warm-up burst
    dum_s = ctx.enter_context(nc.sbuf_tensor("dum_s", [P, 512], dt.bfloat16))

    ps = [
        ctx.enter_context(nc.psum_tensor(f"ps{i}", [P, 512], dt.float32))
        for i in range(4)  # (m,n): 00,01,10,11
    ]
    ps_warm = ctx.enter_context(nc.psum_tensor("ps_warm", [P, 512], dt.float32))

    sems = {}

    def sem(name):
        sems[name] = ctx.enter_context(nc.semaphore(name))
        return sems[name]

    # Per-group completion sems: a DMA's +16 lands as 16 per-spray-engine
    # increments which interleave across in-flight DMAs, so a single counting
    # semaphore is NOT group-ordered. Each spray engine is FIFO, so
    # s_wg[g] >= 16 also implies every earlier group is complete.
    s_wg = [sem(f"s_wg{g}") for g in range(len(W_GROUPS))]
    s_xq = [sem(f"s_xq{i}") for i in range(len(X_GROUPS))]
    s_cst = sem("s_cst")  # ws + bs + asc, +16 each
    s_ps = sem("s_ps")    # tile accumulation complete, +1 per tile in order
    s_dve = sem("s_dve")  # DVE same-engine RAW ordering
    s_ep = sem("s_ep")    # epilogue complete, +1 per tile in order
    s_out = sem("s_out")  # output stores

    TILES = [(0, 0), (0, 1), (1, 0), (1, 1)]

    # Issue the critical first DMAs before anything else: the DGE spin-up
    # takes ~3us and runs while the program clears semaphores / warms up.
    # Prior executions fully drained (Block exit drains), so clearing the
    # semaphores below without a dma_reset is safe.
    nc.sync.dma_start(
        w_s[:, 0 : W_GROUPS[0], :], qw_d.ap()[:, 0 : W_GROUPS[0], :]
    ).then_inc(s_wg[0], 16)
    nc.scalar.dma_start(
        xq_s[:, 0 : X_GROUPS[0], :], xq_d.ap()[:, 0 : X_GROUPS[0], :]
    ).then_inc(s_xq[0], 16)

    # Zero our semaphores up front (a previous execution of this NEFF leaves
    # them at their final values), then barrier so no engine runs ahead.
    nums = sorted(s.num for s in sems.values())
    lo = 0
    while lo < len(nums):
        hi = lo
        while hi + 1 < len(nums) and nums[hi + 1] == nums[hi] + 1:
            hi += 1
        nc.gpsimd.sem_clear(range(nums[lo], nums[hi] + 1))
        lo = hi + 1
    nc.all_engine_barrier()

    w_starts = np.cumsum([0] + W_GROUPS).tolist()
    x_starts = np.cumsum([0] + X_GROUPS).tolist()

    with nc.Block() as block:

        @block.sync
        def _(sync):
            for g in range(1, len(W_GROUPS)):
                c0 = w_starts[g]
                sync.dma_start(
                    w_s[:, c0 : w_starts[g + 1], :],
                    qw_d.ap()[:, c0 : w_starts[g + 1], :],
                ).then_inc(s_wg[g], 16)
            for idx, (m, n) in enumerate(TILES):
                sync.wait_ge(s_ep, idx + 1)
                sync.dma_start(
                    out_d.ap()[m][:, 512 * n : 512 * (n + 1)], o_s[:, idx, :]
                ).then_inc(s_out, 16)

        @block.scalar
        def _(scalar):
            for i in range(1, len(X_GROUPS)):
                c0 = x_starts[i]
                scalar.dma_start(
                    xq_s[:, c0 : x_starts[i + 1], :],
                    xq_d.ap()[:, c0 : x_starts[i + 1], :],
                ).then_inc(s_xq[i], 16)
            scalar.dma_start(as_s[:], as_d.ap()).then_inc(s_cst, 16)
            scalar.dma_start(ws_s[:], ws_d.ap()).then_inc(s_cst, 16)
            scalar.dma_start(bs_s[:], bs_d.ap()).then_inc(s_cst, 16)

        @block.tensor
        def _(tensor):
            # Warm-up burst: garbage matmuls into a scratch PSUM bank keep
            # the PE busy through the HAM activity window while the first
            # real chunks stream in.
            for _ in range(NWARM):
                tensor.matmul(
                    ps_warm.ap(), dum_s[:, 0:P], dum_s[:], start=True, stop=True
                )

            cur_wg = -1
            cur_xp = -1

            def need(c):
                nonlocal cur_wg, cur_xp
                g = _group_of(c, W_GROUPS)
                if g > cur_wg:
                    tensor.wait_ge(s_wg[g], 16)
                    cur_wg = g
                i = _group_of(c, X_GROUPS)
                if i > cur_xp:
                    tensor.wait_ge(s_xq[i], 16)
                    cur_xp = i

            def mm(c, m, n, stop=False, inc=False):
                inst = tensor.matmul(
                    ps[2 * m + n].ap(),
                    xq_s[:, c, P * m : P * (m + 1)],
                    w_s[:, c, 512 * n : 512 * (n + 1)],
                    start=(c == 0),
                    stop=stop,
                )
                if inc:
                    inst.then_inc(s_ps, 1)

            # Phase 1: chunks 0..PH1-1, all 4 tiles per chunk (follows DMA)
            for c in range(PH1):
                need(c)
                for m in range(2):
                    for n in range(2):
                        mm(c, m, n)

            # Phase 2: chunks PH1..KC-1 tile-by-tile; each tile's epilogue
            # and store overlap the next tile's matmuls.
            need(KC - 1)
            for m, n in TILES:
                for c in range(PH1, KC):
                    last = c == KC - 1
                    mm(c, m, n, stop=last, inc=last)

        @block.vector
        def _(vector):
            vector.wait_ge(s_cst, 48)
            for idx, (m, n) in enumerate(TILES):
                nsl = slice(512 * n, 512 * (n + 1))
                vector.wait_ge(s_ps, idx + 1)
                vector.scalar_tensor_tensor(
                    t_s[:, idx, :],
                    ps[2 * m + n].ap(),
                    as_s[:, m : m + 1],
                    ws_s[:, nsl],
                    mybir.AluOpType.mult,
                    mybir.AluOpType.mult,
                ).then_inc(s_dve, 1)
                # DVE is deeply pipelined: same-engine RAW needs a sem
                vector.wait_ge(s_dve, idx + 1)
                vector.tensor_add(
                    o_s[:, idx, :], t_s[:, idx, :], bs_s[:, nsl]
                ).then_inc(s_ep, 1)

    return nc, ctx


def _prep_inputs(x, qweight, w_scales, bias):
    bf16 = cdt.np(mybir.dt.bfloat16)
    fp8 = cdt.np(mybir.dt.float8e4)

    x2 = np.asarray(x, dtype=np.float32).reshape(TOK, K)
    max_abs = np.max(np.abs(x2), axis=-1, keepdims=True)
    act_scales = np.maximum(max_abs / np.float32(127.0), np.float32(EPS)).astype(
        np.float32
    )
    x_q = np.clip(np.round(x2 / act_scales), -127, 127).astype(np.float32)

    # [TOK, K] -> K-major [P, KC, TOK]: xq[p, c, t] = x_q[t, c*128 + p]
    xq = np.ascontiguousarray(
        x_q.T.reshape(KC, P, TOK).transpose(1, 0, 2).astype(bf16)
    )

    # act_scales arranged per m-tile: asc[p, m] = act_scales[m*128 + p]
    asc = np.ascontiguousarray(act_scales.reshape(2, P).T.astype(np.float32))

    # int4-range weights are exactly representable in fp8 e4m3
    qw8 = np.asarray(qweight, dtype=np.int8).astype(fp8)
    w_scales = np.asarray(w_scales, dtype=np.float32)
    bias = np.asarray(bias, dtype=np.float32)

    in_maps = []
    for i in range(N_CORES):
        sl = slice(i * NL, (i + 1) * NL)
        # [K, NL] -> p-major [P, KC, NL]: qw[p, c, n] = shard[c*128 + p, n]
        shard = qw8[:, sl].reshape(KC, P, NL).transpose(1, 0, 2)
        in_maps.append(
            {
                "xq": xq,
                "qw": np.ascontiguousarray(shard),
                "ws": np.ascontiguousarray(
                    np.broadcast_to(w_scales[sl][None, :], (P, NL))
                ),
                "bs": np.ascontiguousarray(
                    np.broadcast_to(bias[sl][None, :], (P, NL))
                ),
                "asc": asc,
            }
        )
    return in_maps


def kernel(x, qweight, w_scales, bias):
    global _cached
    if _cached is None:
        _cached = _build_nc()
    nc, _ = _cached

    in_maps = _prep_inputs(x, qweight, w_scales, bias)
    res = None
    err = None
    for _ in range(3):  # retry transient device errors
        try:
            res = bass_utils.run_bass_kernel_spmd(
                nc, in_maps, core_ids=list(range(N_CORES))
            )
            break
        except Exception as e:  # noqa: BLE001
            err = e
    if res is None:
        raise err

    out = np.empty((TOK, N), dtype=np.float32)
    for i in range(N_CORES):
        out[:, i * NL : (i + 1) * NL] = res.results[i]["out"].reshape(TOK, NL)
    return out.reshape(B, S, N)
